# revision 10
# baseline (speedup 1.0000x reference)
"""LlamaAttention forward on 8 Trainium2 NeuronCores (Bass/Tile), v3.

Sharding: core = b * 4 + g  (b = batch 0/1, g = head-group 0..3, 4 heads each).
Each core computes q/k/v projections + RoPE + causal attention for its 4 heads
of its batch, then a partial output projection against its slice of wo.
The host sums the 4 partial outputs per batch (exact, fp64 accumulate).

vs v1 baseline:
  - All matmul operands bf16 (fp32 PSUM accumulate): halves HBM traffic and
    SBUF footprint; measured metric vs fp32 reference ~3.0e-3 (limit 2e-2).
  - q/k/v stay SBUF-resident between projection and attention (no DRAM
    spill/reload round-trip).
  - Projection s-chunks are 512 wide (LDWEIGHTS fully hidden under N=512
    moving streams; N=256 exposed it).
  - Softmax denominator via an all-ones [128,128] stationary matmul into a
    [128,512] PSUM (every partition holds the column sums), removing the
    reciprocal-broadcast matmul and its copy.
  - combine: full-width (non-diagonal) e tiles are summed 4-at-a-time on the
    DVE (bf16 2x) so one denominator matmul covers four k-tiles (-72 matmuls
    + ldweights; ~-10us/iter measured).
  - Causal diagonal blocks compute only the valid column range (off = j*128)
    in scores / exp / pv / sums.
  - scores(t) issue runs 2 tiles ahead of pv(t)/sums(t) so the ACT exp
    latency never stalls the PE.
  - fuse: attention chunk c is issued right after projection chunk c
    (dataflow matches: attention c needs q chunk c and k/v chunks <= c),
    removing the phase boundary and spreading ACT/DVE load.

Layout notes (per core, S=2048, H=2048, M=512 head-width):
  xT   [H, S]  bf16 hidden.T       -> rhs of q/k projections, lhsT of v proj
  wqT  [H, M]  bf16 (wq/sqrt(hd)).T-> lhsT of q proj (scores scale folded in)
  qT/kT [d, h, S] bf16 SBUF        -> RoPE'd; lhsT/rhs of scoresT = k @ q.T
  scoresT [sk, sq] (psum)          -> +diag mask, exp (ACT) -> e bf16
  pv   [d, sq] psum += v.T @ e     -> v [sk, d] is lhsT, e moving
  sums [*, sq] psum += ones @ e    -> all partitions hold column sums
  attn = pv * 1/sums (DVE)         -> bf16, lhsT of out = attn @ wo_slice.T
"""

import numpy as np
from contextlib import ExitStack

import ml_dtypes

import concourse.bacc as bacc
import concourse.tile as tile
import concourse.mybir as mybir
from concourse.bass_utils import run_bass_kernel_spmd

F32 = mybir.dt.float32
BF16 = mybir.dt.bfloat16
EXP = mybir.ActivationFunctionType.Exp
COPY = mybir.ActivationFunctionType.Copy
MULT = mybir.AluOpType.mult

NCORES = 8
B = 2
HD = 128
NEG = -1.0e30

# filled in by kernel() / test harness
LAST_RESULTS = None


def _build(S, H, M, causal, n_cores=NCORES, repeat=1, combine=True,
           fuse=True):
    """Build + compile the per-core program. M = heads_per_core * 128."""
    P = 128
    NKO = H // P          # h-tiles (contraction) for projections
    NMT = M // P          # heads per core
    QC = 512              # qkv s-chunk (moving free dim)
    NQC = S // QC
    SC = 512              # attention sq-chunk
    NSC = S // SC
    NST = S // P          # sk tiles
    DJ = SC // P          # diagonal sub-tiles per chunk
    assert QC == SC and NQC == NSC

    nc = bacc.Bacc("TRN2", target_bir_lowering=False, debug=False,
                   num_devices=n_cores)

    xT = nc.dram_tensor("xT", [H, S], BF16, kind="ExternalInput").ap()
    wqT = nc.dram_tensor("wqT", [H, M], BF16, kind="ExternalInput").ap()
    wkT = nc.dram_tensor("wkT", [H, M], BF16, kind="ExternalInput").ap()
    wvT = nc.dram_tensor("wvT", [H, M], BF16, kind="ExternalInput").ap()
    woT = nc.dram_tensor("woT", [M, H], BF16, kind="ExternalInput").ap()
    trig = nc.dram_tensor("trig", [P, 2, S], F32, kind="ExternalInput").ap()
    if causal:
        diag_d = nc.dram_tensor("diag", [P, DJ, SC], F32,
                                kind="ExternalInput").ap()
    else:
        maskT_d = nc.dram_tensor("maskT", [S, S], F32, kind="ExternalInput").ap()
    outp = nc.dram_tensor("outp", [S, H], F32, kind="ExternalOutput").ap()

    xT_r = xT.rearrange("(ko p) s -> p ko s", p=P)
    wq_r = wqT.rearrange("(ko p) m -> p ko m", p=P)
    wk_r = wkT.rearrange("(ko p) m -> p ko m", p=P)
    wv_r = wvT.rearrange("(ko p) m -> p ko m", p=P)
    wo_r = woT.rearrange("(mt p) o -> p mt o", p=P)

    with tile.TileContext(nc) as tc, ExitStack() as top, \
         nc.allow_low_precision(reason="bf16 operands; fp32 PSUM accumulate"):
        persist = top.enter_context(tc.tile_pool(name="persist", bufs=1))
        ps_mm = top.enter_context(tc.tile_pool(name="ps_mm", bufs=5, space="PSUM"))
        ps_pv = top.enter_context(tc.tile_pool(name="ps_pv", bufs=2, space="PSUM"))
        ps_sm = top.enter_context(tc.tile_pool(name="ps_sm", bufs=1, space="PSUM"))

        qT_sb = persist.tile([P, NMT, S], BF16)   # [d, head, s]
        kT_sb = persist.tile([P, NMT, S], BF16)
        v_sb = persist.tile([P, NST, M], BF16)    # [sk%128, sk//128, m]
        ones_f = persist.tile([P, P], F32)
        ones_sb = persist.tile([P, P], BF16)      # lhsT for column sums
        if causal:
            diag_sb = persist.tile([P, DJ, SC], F32)

        for _rep in range(repeat):
            nc.vector.memset(ones_f[:], 1.0)
            nc.vector.tensor_copy(ones_sb[:], ones_f[:])

            with tc.tile_pool(name="wq", bufs=1) as wqp, \
                 tc.tile_pool(name="wk", bufs=1) as wkp, \
                 tc.tile_pool(name="wv", bufs=1) as wvp, \
                 tc.tile_pool(name="xc", bufs=2) as xcp, \
                 tc.tile_pool(name="trig", bufs=1) as trigp, \
                 tc.tile_pool(name="ropetmp", bufs=1) as rtp, \
                 tc.tile_pool(name="wo", bufs=1) as wop, \
                 tc.tile_pool(name="exp", bufs=3) as epp, \
                 tc.tile_pool(name="smx", bufs=2) as smp, \
                 tc.tile_pool(name="attc", bufs=2) as attp, \
                 tc.tile_pool(name="oout", bufs=1) as oop, \
                 (tc.tile_pool(name="mskt", bufs=3) if not causal
                  else ExitStack()) as mtp:

                wq_sb = wqp.tile([P, NKO, M], BF16)
                wk_sb = wkp.tile([P, NKO, M], BF16)
                wv_sb = wvp.tile([P, NKO, M], BF16)
                wo_sb = wop.tile([P, NMT, H], BF16)
                xcs = {}
                trigs = {}

                def load_trig(c):
                    trigs[c] = trigp.tile([P, 2, QC], F32, tag="trig", name="tg")
                    nc.sync.dma_start(trigs[c][:], trig[:, :, c * QC:(c + 1) * QC])

                def load_xc(c, engine=None):
                    xcs[c] = xcp.tile([P, NKO, QC], BF16, tag="xc", name="xc")
                    (engine or nc.sync).dma_start(
                        xcs[c][:], xT_r[:, :, c * QC:(c + 1) * QC])
                    if c not in trigs:
                        load_trig(c)

                def qk_proj(c):
                    tg = trigs.pop(c)
                    cs = slice(c * QC, (c + 1) * QC)
                    for w_sb, dst in ((wq_sb, qT_sb), (wk_sb, kT_sb)):
                        for mt in range(NMT):
                            ps = ps_mm.tile([P, QC], F32, tag="mm", name="ps_qk")
                            for ko in range(NKO):
                                nc.tensor.matmul(
                                    ps, w_sb[:, ko, mt * P:(mt + 1) * P],
                                    xcs[c][:, ko, :],
                                    start=(ko == 0), stop=(ko == NKO - 1))
                            # RoPE: out = ps*cos + swap64(ps)*sin_signed
                            t1 = rtp.tile([P, QC], F32, tag="t1")
                            nc.vector.tensor_tensor(t1[:], ps, tg[:, 0, :], MULT)
                            t2 = rtp.tile([P, QC], F32, tag="t2")
                            nc.vector.tensor_tensor(t2[0:64, :], ps[64:128, :],
                                                    tg[0:64, 1, :], MULT)
                            nc.vector.tensor_tensor(t2[64:128, :], ps[0:64, :],
                                                    tg[64:128, 1, :], MULT)
                            nc.vector.tensor_add(dst[:, mt, cs], t1[:], t2[:])

                def v_proj(c):
                    for st in range(QC // P):
                        ps = ps_mm.tile([P, M], F32, tag="mm", name="ps_v")
                        for ko in range(NKO):
                            nc.tensor.matmul(
                                ps, xcs[c][:, ko, st * P:(st + 1) * P],
                                wv_sb[:, ko, :],
                                start=(ko == 0), stop=(ko == NKO - 1))
                        nc.scalar.activation(
                            v_sb[:, c * (QC // P) + st, :], ps, COPY)

                def attn_chunk(c):
                    cq = slice(c * SC, (c + 1) * SC)
                    nt = min(NST, (c + 1) * SC // P) if causal else NST
                    attn_c = attp.tile([P, NMT, SC], BF16, tag="attc",
                                       name="attn_c")
                    for h in range(NMT):
                        hs = slice(h * P, (h + 1) * P)
                        pv = ps_pv.tile([P, SC], F32, tag="pv")
                        sm = ps_sm.tile([P, SC], F32, tag="sums")
                        ndiag = c * DJ if (causal and combine) else 0
                        first_sum = [True]

                        def issue_scores(t, e_dst):
                            j = t - c * DJ
                            off = j * P if (causal and j >= 0) else 0
                            ss_full = ps_mm.tile([P, SC], F32, tag="mm")
                            ss = ss_full[:, off:]
                            nc.tensor.matmul(
                                ss, kT_sb[:, h, t * P:(t + 1) * P],
                                qT_sb[:, h, c * SC + off:(c + 1) * SC],
                                start=True, stop=True)
                            if causal:
                                if j >= 0:
                                    we = (j + 1) * P
                                    nc.vector.tensor_add(
                                        ss_full[:, off:we], ss_full[:, off:we],
                                        diag_sb[:, j, off:we])
                            else:
                                mt_t = mtp.tile([P, SC], F32, tag="mask")
                                nc.sync.dma_start(
                                    mt_t[:], maskT_d[t * P:(t + 1) * P, cq])
                                nc.vector.tensor_add(ss, ss, mt_t[:, off:])
                                nc.vector.tensor_scalar_max(ss, ss, -3.0e38)
                            nc.scalar.activation(e_dst[:, off:], ss, EXP)
                            return (t, off, e_dst)

                        def flush_pv(t, off, e):
                            nc.tensor.matmul(
                                pv[:, off:], v_sb[:, t, hs], e[:, off:],
                                start=(t == 0), stop=(t == nt - 1),
                                skip_group_check=True)

                        def issue_sum(off, rhs, is_last):
                            nc.tensor.matmul(
                                sm[:, off:], ones_sb[:], rhs[:, off:],
                                start=first_sum[0], stop=is_last,
                                skip_group_check=True)
                            first_sum[0] = False

                        pend = []
                        sums_ready = []
                        equad = None
                        for t in range(nt):
                            if t < ndiag:
                                gi = t % 4
                                if gi == 0:
                                    equad = epp.tile([P, 4, SC], BF16,
                                                     tag="e4")
                                pend.append(issue_scores(t, equad[:, gi, :]))
                                if gi == 3:
                                    e2 = epp.tile([P, 2, SC], BF16, tag="e2")
                                    nc.vector.tensor_add(
                                        e2[:], equad[:, 0:2, :],
                                        equad[:, 2:4, :])
                                    e1 = epp.tile([P, SC], BF16, tag="e1")
                                    nc.vector.tensor_add(
                                        e1[:], e2[:, 0, :], e2[:, 1, :])
                                    sums_ready.append((0, e1, False))
                            else:
                                e = epp.tile([P, SC], BF16, tag="e")
                                res = issue_scores(t, e)
                                pend.append(res)
                                sums_ready.append((res[1], e, t == nt - 1))
                            if len(pend) > 2:
                                flush_pv(*pend.pop(0))
                            if len(sums_ready) > 1:
                                issue_sum(*sums_ready.pop(0))
                        for args in pend:
                            flush_pv(*args)
                        for s in sums_ready:
                            issue_sum(*s)

                        rec = smp.tile([P, SC], F32, tag="rec")
                        nc.vector.reciprocal(rec[:], sm)
                        nc.vector.tensor_tensor(attn_c[:, h, :], pv, rec[:],
                                                MULT)

                    # o-proj for this sq chunk (all heads of attn_c ready)
                    for st in range(SC // P):
                        rs = slice(c * SC + st * P, c * SC + (st + 1) * P)
                        o_st = oop.tile([P, H], F32, tag="o_st", name="o_st")
                        for oc in range(H // SC):
                            ps = ps_mm.tile([P, SC], F32, tag="mm", name="ps_o")
                            for mt in range(NMT):
                                nc.tensor.matmul(
                                    ps, attn_c[:, mt, st * P:(st + 1) * P],
                                    wo_sb[:, mt, oc * SC:(oc + 1) * SC],
                                    start=(mt == 0), stop=(mt == NMT - 1))
                            nc.scalar.activation(
                                o_st[:, oc * SC:(oc + 1) * SC], ps, COPY)
                            if oc % 2 == 1:  # flush staged half early
                                hs_o = slice((oc - 1) * SC, (oc + 1) * SC)
                                nc.sync.dma_start(outp[rs, hs_o],
                                                  o_st[:, hs_o])

                # DMA order: x chunk 0, wq per head-slice (first q chain can
                # start early), trig0, wk, then wv/wo + x1+ stream under
                # compute.
                load_xc(0)
                for mt in range(NMT):
                    ms = slice(mt * P, (mt + 1) * P)
                    nc.sync.dma_start(wq_sb[:, :, ms], wq_r[:, :, ms])
                load_trig(0)
                for mt in range(NMT):
                    ms = slice(mt * P, (mt + 1) * P)
                    nc.sync.dma_start(wk_sb[:, :, ms], wk_r[:, :, ms])
                if causal:
                    nc.scalar.dma_start(diag_sb[:], diag_d[:])

                # c=0/1 front: q,k first (wv still in flight), then v(0), v(1)
                qk_proj(0)
                load_xc(1)
                nc.sync.dma_start(wv_sb[:], wv_r)
                qk_proj(1)
                v_proj(0)
                v_proj(1)
                del xcs[0]
                for mt in range(NMT):
                    nc.scalar.dma_start(wo_sb[:, mt], wo_r[:, mt])

                if fuse:
                    # attn chunk c right after its projections; projections of
                    # chunk c+1 interleave with attention/o-proj of chunk c-1.
                    attn_chunk(0)
                    for c in range(2, NQC):
                        load_xc(c)
                        qk_proj(c)
                        v_proj(c)
                        del xcs[c - 1]
                        attn_chunk(c - 1)
                    attn_chunk(NQC - 1)
                else:
                    for c in range(2, NQC):
                        load_xc(c)
                        qk_proj(c)
                        v_proj(c)
                        del xcs[c - 1]
                    for c in range(NSC):
                        attn_chunk(c)

    nc.compile()
    return nc


_CACHE = {}


def _get_program(S, H, M, causal, repeat=1, combine=True, fuse=True):
    key = (S, H, M, causal, repeat, combine, fuse)
    if key not in _CACHE:
        _CACHE[key] = _build(S, H, M, causal, repeat=repeat, combine=combine,
                             fuse=fuse)
    return _CACHE[key]


def _rope_tables(S, dim, base=10000.0):
    inv_freq = 1.0 / (base ** (np.arange(0, dim, 2, dtype=np.float64) / dim))
    t = np.arange(S, dtype=np.float64)
    freqs = np.outer(t, inv_freq)                     # [S, dim/2]
    emb = np.concatenate([freqs, freqs], axis=-1)     # [S, dim]
    return (np.cos(emb).astype(np.float32), np.sin(emb).astype(np.float32))


def _prep_in_maps(hidden_states, attention_mask, position_ids,
                  wq, wk, wv, wo):
    """Host-side shard + cast. Returns (in_maps, causal, M)."""
    Bq, S, H = hidden_states.shape
    assert Bq == B and H % HD == 0
    nh = H // HD
    groups = NCORES // B                     # head-groups per batch
    hpg = nh // groups                       # heads per core
    M = hpg * HD

    bf = ml_dtypes.bfloat16

    # causal-mask detection (exact match against the standard Llama pattern)
    neg = np.finfo(np.float32).min
    causal_ref = np.where(np.tril(np.ones((S, S), dtype=bool)),
                          np.float32(0.0), np.float32(neg))
    causal = all(np.array_equal(attention_mask[b, 0], causal_ref)
                 for b in range(B))

    cos_tab, sin_tab = _rope_tables(S, HD)
    scale = 1.0 / np.sqrt(HD)

    SC = 512
    P = 128
    in_maps = []
    for core in range(NCORES):
        b, g = divmod(core, groups)
        rows = slice(g * M, (g + 1) * M)
        x = hidden_states[b]                                   # [S, H]
        pos = position_ids[b].astype(np.int64)
        cosT = cos_tab[pos].T                                  # [HD, S]
        sinT = sin_tab[pos].T
        sinS = np.concatenate([-sinT[:HD // 2], sinT[HD // 2:]], axis=0)
        m = {
            "xT": np.ascontiguousarray(x.T).astype(bf),
            "wqT": np.ascontiguousarray((wq[rows].astype(np.float64) * scale)
                                        .astype(np.float32).T).astype(bf),
            "wkT": np.ascontiguousarray(wk[rows].T).astype(bf),
            "wvT": np.ascontiguousarray(wv[rows].T).astype(bf),
            "woT": np.ascontiguousarray(wo[:, rows].T).astype(bf),
            "trig": np.ascontiguousarray(
                np.stack([cosT, sinS], axis=1).astype(np.float32)),
        }
        if causal:
            p_idx = np.arange(P)[:, None, None]
            j_idx = np.arange(SC // P)[None, :, None]
            f_idx = np.arange(SC)[None, None, :]
            m["diag"] = np.where(p_idx + P * j_idx <= f_idx,
                                 np.float32(0.0),
                                 np.float32(NEG)).astype(np.float32)
        else:
            m["maskT"] = np.ascontiguousarray(attention_mask[b, 0].T)
        in_maps.append(m)
    return in_maps, causal, M


def kernel(hidden_states, attention_mask, position_ids, wq, wk, wv, wo):
    global LAST_RESULTS
    hidden_states = np.asarray(hidden_states, dtype=np.float32)
    attention_mask = np.asarray(attention_mask, dtype=np.float32)
    position_ids = np.asarray(position_ids)
    wq = np.asarray(wq, dtype=np.float32)
    wk = np.asarray(wk, dtype=np.float32)
    wv = np.asarray(wv, dtype=np.float32)
    wo = np.asarray(wo, dtype=np.float32)

    Bq, S, H = hidden_states.shape
    in_maps, causal, M = _prep_in_maps(
        hidden_states, attention_mask, position_ids, wq, wk, wv, wo)

    nc = _get_program(S, H, M, causal)
    globals()["LAST_IN_MAPS"] = in_maps
    res = run_bass_kernel_spmd(nc, in_maps, core_ids=list(range(NCORES)),
                               trace=False)
    LAST_RESULTS = res

    groups = NCORES // B
    out = np.zeros((B, S, H), dtype=np.float64)
    for core in range(NCORES):
        b = core // groups
        out[b] += res.results[core]["outp"].astype(np.float64)
    return out.astype(np.float32)


# revision 12
# speedup vs baseline: 1.0194x; 1.0194x over previous
"""LlamaAttention forward on 8 Trainium2 NeuronCores (Bass/Tile), v3.

Sharding: core = b * 4 + g  (b = batch 0/1, g = head-group 0..3, 4 heads each).
Each core computes q/k/v projections + RoPE + causal attention for its 4 heads
of its batch, then a partial output projection against its slice of wo.
The host sums the 4 partial outputs per batch (exact, fp64 accumulate).

vs v1 baseline:
  - All matmul operands bf16 (fp32 PSUM accumulate): halves HBM traffic and
    SBUF footprint; measured metric vs fp32 reference ~3.0e-3 (limit 2e-2).
  - q/k/v stay SBUF-resident between projection and attention (no DRAM
    spill/reload round-trip).
  - Projection s-chunks are 512 wide (LDWEIGHTS fully hidden under N=512
    moving streams; N=256 exposed it).
  - Softmax denominator via an all-ones [128,128] stationary matmul into a
    [128,512] PSUM (every partition holds the column sums), removing the
    reciprocal-broadcast matmul and its copy.
  - combine: full-width (non-diagonal) e tiles are summed 4-at-a-time on the
    DVE (bf16 2x) so one denominator matmul covers four k-tiles (-72 matmuls
    + ldweights; ~-10us/iter measured).
  - Causal diagonal blocks compute only the valid column range (off = j*128)
    in scores / exp / pv / sums.
  - scores(t) issue runs 3 tiles ahead of pv(t)/sums(t) so the ACT exp
    latency never stalls the PE.
  - fuse: attention chunk c is issued right after projection chunk c
    (dataflow matches: attention c needs q chunk c and k/v chunks <= c),
    removing the phase boundary and spreading ACT/DVE load.

Layout notes (per core, S=2048, H=2048, M=512 head-width):
  xT   [H, S]  bf16 hidden.T       -> rhs of q/k projections, lhsT of v proj
  wqT  [H, M]  bf16 (wq/sqrt(hd)).T-> lhsT of q proj (scores scale folded in)
  qT/kT [d, h, S] bf16 SBUF        -> RoPE'd; lhsT/rhs of scoresT = k @ q.T
  scoresT [sk, sq] (psum)          -> +diag mask, exp (ACT) -> e bf16
  pv   [d, sq] psum += v.T @ e     -> v [sk, d] is lhsT, e moving
  sums [*, sq] psum += ones @ e    -> all partitions hold column sums
  attn = pv * 1/sums (DVE)         -> bf16, lhsT of out = attn @ wo_slice.T
"""

import numpy as np
from contextlib import ExitStack

import ml_dtypes

import concourse.bacc as bacc
import concourse.tile as tile
import concourse.mybir as mybir
from concourse.bass_utils import run_bass_kernel_spmd

F32 = mybir.dt.float32
BF16 = mybir.dt.bfloat16
EXP = mybir.ActivationFunctionType.Exp
COPY = mybir.ActivationFunctionType.Copy
MULT = mybir.AluOpType.mult

NCORES = 8
B = 2
HD = 128
NEG = -1.0e30

# filled in by kernel() / test harness
LAST_RESULTS = None


def _build(S, H, M, causal, n_cores=NCORES, repeat=1, combine=True,
           fuse=True):
    """Build + compile the per-core program. M = heads_per_core * 128."""
    P = 128
    NKO = H // P          # h-tiles (contraction) for projections
    NMT = M // P          # heads per core
    QC = 512              # qkv s-chunk (moving free dim)
    NQC = S // QC
    SC = 512              # attention sq-chunk
    NSC = S // SC
    NST = S // P          # sk tiles
    DJ = SC // P          # diagonal sub-tiles per chunk
    assert QC == SC and NQC == NSC

    nc = bacc.Bacc("TRN2", target_bir_lowering=False, debug=False,
                   num_devices=n_cores)

    xT = nc.dram_tensor("xT", [H, S], BF16, kind="ExternalInput").ap()
    wqT = nc.dram_tensor("wqT", [H, M], BF16, kind="ExternalInput").ap()
    wkT = nc.dram_tensor("wkT", [H, M], BF16, kind="ExternalInput").ap()
    wvT = nc.dram_tensor("wvT", [H, M], BF16, kind="ExternalInput").ap()
    woT = nc.dram_tensor("woT", [M, H], BF16, kind="ExternalInput").ap()
    trig = nc.dram_tensor("trig", [P, 2, S], F32, kind="ExternalInput").ap()
    if causal:
        diag_d = nc.dram_tensor("diag", [P, DJ, SC], BF16,
                                kind="ExternalInput").ap()
    else:
        maskT_d = nc.dram_tensor("maskT", [S, S], F32, kind="ExternalInput").ap()
    outp = nc.dram_tensor("outp", [S, H], F32, kind="ExternalOutput").ap()

    xT_r = xT.rearrange("(ko p) s -> p ko s", p=P)
    wq_r = wqT.rearrange("(ko p) m -> p ko m", p=P)
    wk_r = wkT.rearrange("(ko p) m -> p ko m", p=P)
    wv_r = wvT.rearrange("(ko p) m -> p ko m", p=P)
    wo_r = woT.rearrange("(mt p) o -> p mt o", p=P)

    with tile.TileContext(nc) as tc, ExitStack() as top, \
         nc.allow_low_precision(reason="bf16 operands; fp32 PSUM accumulate"):
        persist = top.enter_context(tc.tile_pool(name="persist", bufs=1))
        ps_mm = top.enter_context(tc.tile_pool(name="ps_mm", bufs=5, space="PSUM"))
        ps_pv = top.enter_context(tc.tile_pool(name="ps_pv", bufs=2, space="PSUM"))
        ps_sm = top.enter_context(tc.tile_pool(name="ps_sm", bufs=1, space="PSUM"))

        qT_sb = persist.tile([P, NMT, S], BF16)   # [d, head, s]
        kT_sb = persist.tile([P, NMT, S], BF16)
        v_sb = persist.tile([P, NST, M], BF16)    # [sk%128, sk//128, m]
        ones_sb = persist.tile([P, P], BF16)      # lhsT for column sums
        if causal:
            diag_sb = persist.tile([P, DJ, SC], BF16)

        for _rep in range(repeat):
            nc.vector.memset(ones_sb[:], 1.0)

            with tc.tile_pool(name="wq", bufs=1) as wqp, \
                 tc.tile_pool(name="wk", bufs=1) as wkp, \
                 tc.tile_pool(name="wv", bufs=1) as wvp, \
                 tc.tile_pool(name="xc", bufs=2) as xcp, \
                 tc.tile_pool(name="trig", bufs=1) as trigp, \
                 tc.tile_pool(name="ropetmp", bufs=1) as rtp, \
                 tc.tile_pool(name="wo", bufs=1) as wop, \
                 tc.tile_pool(name="exp", bufs=4) as epp, \
                 tc.tile_pool(name="smx", bufs=1) as smp, \
                 tc.tile_pool(name="attc", bufs=2) as attp, \
                 tc.tile_pool(name="oout", bufs=1) as oop, \
                 (tc.tile_pool(name="mskt", bufs=3) if not causal
                  else ExitStack()) as mtp:

                wq_sb = wqp.tile([P, NKO, M], BF16)
                wk_sb = wkp.tile([P, NKO, M], BF16)
                wv_sb = wvp.tile([P, NKO, M], BF16)
                wo_sb = wop.tile([P, NMT, H], BF16)
                xcs = {}
                trigs = {}

                def load_trig(c):
                    trigs[c] = trigp.tile([P, 2, QC], F32, tag="trig", name="tg")
                    nc.sync.dma_start(trigs[c][:], trig[:, :, c * QC:(c + 1) * QC])

                def load_xc(c, engine=None):
                    xcs[c] = xcp.tile([P, NKO, QC], BF16, tag="xc", name="xc")
                    (engine or nc.sync).dma_start(
                        xcs[c][:], xT_r[:, :, c * QC:(c + 1) * QC])
                    if c not in trigs:
                        load_trig(c)

                def qk_proj(c):
                    tg = trigs.pop(c)
                    cs = slice(c * QC, (c + 1) * QC)
                    for w_sb, dst in ((wq_sb, qT_sb), (wk_sb, kT_sb)):
                        for mt in range(NMT):
                            ps = ps_mm.tile([P, QC], F32, tag="mm", name="ps_qk")
                            for ko in range(NKO):
                                nc.tensor.matmul(
                                    ps, w_sb[:, ko, mt * P:(mt + 1) * P],
                                    xcs[c][:, ko, :],
                                    start=(ko == 0), stop=(ko == NKO - 1))
                            # RoPE: out = ps*cos + swap64(ps)*sin_signed
                            t1 = rtp.tile([P, QC], F32, tag="t1")
                            nc.vector.tensor_tensor(t1[:], ps, tg[:, 0, :], MULT)
                            t2 = rtp.tile([P, QC], F32, tag="t2")
                            nc.vector.tensor_tensor(t2[0:64, :], ps[64:128, :],
                                                    tg[0:64, 1, :], MULT)
                            nc.vector.tensor_tensor(t2[64:128, :], ps[0:64, :],
                                                    tg[64:128, 1, :], MULT)
                            nc.vector.tensor_add(dst[:, mt, cs], t1[:], t2[:])

                def v_proj(c):
                    for st in range(QC // P):
                        ps = ps_mm.tile([P, M], F32, tag="mm", name="ps_v")
                        for ko in range(NKO):
                            nc.tensor.matmul(
                                ps, xcs[c][:, ko, st * P:(st + 1) * P],
                                wv_sb[:, ko, :],
                                start=(ko == 0), stop=(ko == NKO - 1))
                        nc.scalar.activation(
                            v_sb[:, c * (QC // P) + st, :], ps, COPY)

                def attn_chunk(c):
                    cq = slice(c * SC, (c + 1) * SC)
                    nt = min(NST, (c + 1) * SC // P) if causal else NST
                    attn_c = attp.tile([P, NMT, SC], BF16, tag="attc",
                                       name="attn_c")
                    for h in range(NMT):
                        hs = slice(h * P, (h + 1) * P)
                        pv = ps_pv.tile([P, SC], F32, tag="pv")
                        sm = ps_sm.tile([P, SC], F32, tag="sums")
                        ndiag = c * DJ if (causal and combine) else 0
                        first_sum = [True]

                        def issue_scores(t, e_dst):
                            j = t - c * DJ
                            off = j * P if (causal and j >= 0) else 0
                            ss_full = ps_mm.tile([P, SC], F32, tag="mm")
                            ss = ss_full[:, off:]
                            nc.tensor.matmul(
                                ss, kT_sb[:, h, t * P:(t + 1) * P],
                                qT_sb[:, h, c * SC + off:(c + 1) * SC],
                                start=True, stop=True)
                            if causal:
                                if j >= 0:
                                    we = (j + 1) * P
                                    nc.vector.tensor_add(
                                        ss_full[:, off:we], ss_full[:, off:we],
                                        diag_sb[:, j, off:we])
                            else:
                                mt_t = mtp.tile([P, SC], F32, tag="mask")
                                nc.sync.dma_start(
                                    mt_t[:], maskT_d[t * P:(t + 1) * P, cq])
                                nc.vector.tensor_add(ss, ss, mt_t[:, off:])
                                nc.vector.tensor_scalar_max(ss, ss, -3.0e38)
                            nc.scalar.activation(e_dst[:, off:], ss, EXP)
                            return (t, off, e_dst)

                        def flush_pv(t, off, e):
                            nc.tensor.matmul(
                                pv[:, off:], v_sb[:, t, hs], e[:, off:],
                                start=(t == 0), stop=(t == nt - 1),
                                skip_group_check=True)

                        def issue_sum(off, rhs, is_last):
                            nc.tensor.matmul(
                                sm[:, off:], ones_sb[:], rhs[:, off:],
                                start=first_sum[0], stop=is_last,
                                skip_group_check=True)
                            first_sum[0] = False

                        pend = []
                        sums_ready = []
                        equad = None
                        for t in range(nt):
                            if t < ndiag:
                                gi = t % 4
                                if gi == 0:
                                    equad = epp.tile([P, 4, SC], BF16,
                                                     tag="e4")
                                pend.append(issue_scores(t, equad[:, gi, :]))
                                if gi == 3:
                                    e2 = epp.tile([P, 2, SC], BF16, tag="e2")
                                    nc.vector.tensor_add(
                                        e2[:], equad[:, 0:2, :],
                                        equad[:, 2:4, :])
                                    e1 = epp.tile([P, SC], BF16, tag="e1")
                                    nc.vector.tensor_add(
                                        e1[:], e2[:, 0, :], e2[:, 1, :])
                                    sums_ready.append((0, e1, False))
                            else:
                                e = epp.tile([P, SC], BF16, tag="e")
                                res = issue_scores(t, e)
                                pend.append(res)
                                sums_ready.append((res[1], e, t == nt - 1))
                            if len(pend) > 3:
                                flush_pv(*pend.pop(0))
                            if len(sums_ready) > 1:
                                issue_sum(*sums_ready.pop(0))
                        for args in pend:
                            flush_pv(*args)
                        for s in sums_ready:
                            issue_sum(*s)

                        rec = smp.tile([P, SC], F32, tag="rec")
                        nc.vector.reciprocal(rec[:], sm)
                        nc.vector.tensor_tensor(attn_c[:, h, :], pv, rec[:],
                                                MULT)

                    # o-proj for this sq chunk (all heads of attn_c ready)
                    for st in range(SC // P):
                        rs = slice(c * SC + st * P, c * SC + (st + 1) * P)
                        o_st = oop.tile([P, H], F32, tag="o_st", name="o_st")
                        for oc in range(H // SC):
                            ps = ps_mm.tile([P, SC], F32, tag="mm", name="ps_o")
                            for mt in range(NMT):
                                nc.tensor.matmul(
                                    ps, attn_c[:, mt, st * P:(st + 1) * P],
                                    wo_sb[:, mt, oc * SC:(oc + 1) * SC],
                                    start=(mt == 0), stop=(mt == NMT - 1))
                            nc.scalar.activation(
                                o_st[:, oc * SC:(oc + 1) * SC], ps, COPY)
                            if oc % 2 == 1:  # flush staged half early
                                hs_o = slice((oc - 1) * SC, (oc + 1) * SC)
                                nc.sync.dma_start(outp[rs, hs_o],
                                                  o_st[:, hs_o])

                # DMA order: x chunk 0, wq per head-slice (first q chain can
                # start early), trig0, wk, then wv/wo + x1+ stream under
                # compute.
                load_xc(0)
                for mt in range(NMT):
                    ms = slice(mt * P, (mt + 1) * P)
                    nc.sync.dma_start(wq_sb[:, :, ms], wq_r[:, :, ms])
                load_trig(0)
                for mt in range(NMT):
                    ms = slice(mt * P, (mt + 1) * P)
                    nc.sync.dma_start(wk_sb[:, :, ms], wk_r[:, :, ms])
                if causal:
                    nc.scalar.dma_start(diag_sb[:], diag_d[:])

                # c=0/1 front: q,k first (wv still in flight), then v(0), v(1)
                qk_proj(0)
                load_xc(1)
                nc.sync.dma_start(wv_sb[:], wv_r)
                qk_proj(1)
                v_proj(0)
                v_proj(1)
                del xcs[0]
                for mt in range(NMT):
                    nc.scalar.dma_start(wo_sb[:, mt], wo_r[:, mt])

                if fuse:
                    # attn chunk c right after its projections; projections of
                    # chunk c+1 interleave with attention/o-proj of chunk c-1.
                    attn_chunk(0)
                    for c in range(2, NQC):
                        load_xc(c)
                        qk_proj(c)
                        v_proj(c)
                        del xcs[c - 1]
                        attn_chunk(c - 1)
                    attn_chunk(NQC - 1)
                else:
                    for c in range(2, NQC):
                        load_xc(c)
                        qk_proj(c)
                        v_proj(c)
                        del xcs[c - 1]
                    for c in range(NSC):
                        attn_chunk(c)

    nc.compile()
    return nc


_CACHE = {}


def _get_program(S, H, M, causal, repeat=1, combine=True, fuse=True):
    key = (S, H, M, causal, repeat, combine, fuse)
    if key not in _CACHE:
        _CACHE[key] = _build(S, H, M, causal, repeat=repeat, combine=combine,
                             fuse=fuse)
    return _CACHE[key]


def _rope_tables(S, dim, base=10000.0):
    inv_freq = 1.0 / (base ** (np.arange(0, dim, 2, dtype=np.float64) / dim))
    t = np.arange(S, dtype=np.float64)
    freqs = np.outer(t, inv_freq)                     # [S, dim/2]
    emb = np.concatenate([freqs, freqs], axis=-1)     # [S, dim]
    return (np.cos(emb).astype(np.float32), np.sin(emb).astype(np.float32))


def _prep_in_maps(hidden_states, attention_mask, position_ids,
                  wq, wk, wv, wo):
    """Host-side shard + cast. Returns (in_maps, causal, M)."""
    Bq, S, H = hidden_states.shape
    assert Bq == B and H % HD == 0
    nh = H // HD
    groups = NCORES // B                     # head-groups per batch
    hpg = nh // groups                       # heads per core
    M = hpg * HD

    bf = ml_dtypes.bfloat16

    # causal-mask detection (exact match against the standard Llama pattern)
    neg = np.finfo(np.float32).min
    causal_ref = np.where(np.tril(np.ones((S, S), dtype=bool)),
                          np.float32(0.0), np.float32(neg))
    causal = all(np.array_equal(attention_mask[b, 0], causal_ref)
                 for b in range(B))

    cos_tab, sin_tab = _rope_tables(S, HD)
    scale = 1.0 / np.sqrt(HD)

    SC = 512
    P = 128
    in_maps = []
    for core in range(NCORES):
        b, g = divmod(core, groups)
        rows = slice(g * M, (g + 1) * M)
        x = hidden_states[b]                                   # [S, H]
        pos = position_ids[b].astype(np.int64)
        cosT = cos_tab[pos].T                                  # [HD, S]
        sinT = sin_tab[pos].T
        sinS = np.concatenate([-sinT[:HD // 2], sinT[HD // 2:]], axis=0)
        m = {
            "xT": np.ascontiguousarray(x.T).astype(bf),
            "wqT": np.ascontiguousarray((wq[rows].astype(np.float64) * scale)
                                        .astype(np.float32).T).astype(bf),
            "wkT": np.ascontiguousarray(wk[rows].T).astype(bf),
            "wvT": np.ascontiguousarray(wv[rows].T).astype(bf),
            "woT": np.ascontiguousarray(wo[:, rows].T).astype(bf),
            "trig": np.ascontiguousarray(
                np.stack([cosT, sinS], axis=1).astype(np.float32)),
        }
        if causal:
            p_idx = np.arange(P)[:, None, None]
            j_idx = np.arange(SC // P)[None, :, None]
            f_idx = np.arange(SC)[None, None, :]
            m["diag"] = np.where(p_idx + P * j_idx <= f_idx,
                                 np.float32(0.0),
                                 np.float32(NEG)).astype(bf)
        else:
            m["maskT"] = np.ascontiguousarray(attention_mask[b, 0].T)
        in_maps.append(m)
    return in_maps, causal, M


def kernel(hidden_states, attention_mask, position_ids, wq, wk, wv, wo):
    global LAST_RESULTS
    hidden_states = np.asarray(hidden_states, dtype=np.float32)
    attention_mask = np.asarray(attention_mask, dtype=np.float32)
    position_ids = np.asarray(position_ids)
    wq = np.asarray(wq, dtype=np.float32)
    wk = np.asarray(wk, dtype=np.float32)
    wv = np.asarray(wv, dtype=np.float32)
    wo = np.asarray(wo, dtype=np.float32)

    Bq, S, H = hidden_states.shape
    in_maps, causal, M = _prep_in_maps(
        hidden_states, attention_mask, position_ids, wq, wk, wv, wo)

    nc = _get_program(S, H, M, causal)
    globals()["LAST_IN_MAPS"] = in_maps
    res = run_bass_kernel_spmd(nc, in_maps, core_ids=list(range(NCORES)),
                               trace=False)
    LAST_RESULTS = res

    groups = NCORES // B
    out = np.zeros((B, S, H), dtype=np.float64)
    for core in range(NCORES):
        b = core // groups
        out[b] += res.results[core]["outp"].astype(np.float64)
    return out.astype(np.float32)


# revision 13
# speedup vs baseline: 1.0554x; 1.0353x over previous
"""LlamaAttention forward on 8 Trainium2 NeuronCores (Bass/Tile), v3.

Sharding: core = b * 4 + g  (b = batch 0/1, g = head-group 0..3, 4 heads each).
Each core computes q/k/v projections + RoPE + causal attention for its 4 heads
of its batch, then a partial output projection against its slice of wo.
The host sums the 4 partial outputs per batch (exact, fp64 accumulate).

vs v1 baseline:
  - All matmul operands bf16 (fp32 PSUM accumulate): halves HBM traffic and
    SBUF footprint; measured metric vs fp32 reference ~3.0e-3 (limit 2e-2).
  - q/k/v stay SBUF-resident between projection and attention (no DRAM
    spill/reload round-trip).
  - Projection s-chunks are 512 wide (LDWEIGHTS fully hidden under N=512
    moving streams; N=256 exposed it).
  - Softmax denominator via an all-ones [128,128] stationary matmul into a
    [128,512] PSUM (every partition holds the column sums), removing the
    reciprocal-broadcast matmul and its copy.
  - combine: full-width (non-diagonal) e tiles are summed 4-at-a-time on the
    DVE (bf16 2x) so one denominator matmul covers four k-tiles (-72 matmuls
    + ldweights; ~-10us/iter measured).
  - Causal diagonal blocks compute only the valid column range (off = j*128)
    in scores / exp / pv / sums.
  - scores(t) issue runs 3 tiles ahead of pv(t)/sums(t) so the ACT exp
    latency never stalls the PE.
  - fuse: attention chunk c is issued right after projection chunk c
    (dataflow matches: attention c needs q chunk c and k/v chunks <= c),
    removing the phase boundary and spreading ACT/DVE load.

Layout notes (per core, S=2048, H=2048, M=512 head-width):
  xT   [H, S]  bf16 hidden.T       -> rhs of q/k projections, lhsT of v proj
  wqT  [H, M]  bf16 (wq/sqrt(hd)).T-> lhsT of q proj (scores scale folded in)
  qT/kT [d, h, S] bf16 SBUF        -> RoPE'd; lhsT/rhs of scoresT = k @ q.T
  scoresT [sk, sq] (psum)          -> +diag mask, exp (ACT) -> e bf16
  pv   [d, sq] psum += v.T @ e     -> v [sk, d] is lhsT, e moving
  sums [*, sq] psum += ones @ e    -> all partitions hold column sums
  attn = pv * 1/sums (DVE)         -> bf16, lhsT of out = attn @ wo_slice.T
"""

import numpy as np
from contextlib import ExitStack

import ml_dtypes

import concourse.bacc as bacc
import concourse.tile as tile
import concourse.mybir as mybir
from concourse.bass_utils import run_bass_kernel_spmd

F32 = mybir.dt.float32
BF16 = mybir.dt.bfloat16
EXP = mybir.ActivationFunctionType.Exp
COPY = mybir.ActivationFunctionType.Copy
MULT = mybir.AluOpType.mult

NCORES = 8
B = 2
HD = 128
NEG = -1.0e30

# filled in by kernel() / test harness
LAST_RESULTS = None


def _build(S, H, M, causal, n_cores=NCORES, repeat=1, combine=True,
           fuse=True):
    """Build + compile the per-core program. M = heads_per_core * 128."""
    P = 128
    NKO = H // P          # h-tiles (contraction) for projections
    NMT = M // P          # heads per core
    QC = 512              # qkv s-chunk (moving free dim)
    NQC = S // QC
    SC = 512              # attention sq-chunk
    NSC = S // SC
    NST = S // P          # sk tiles
    DJ = SC // P          # diagonal sub-tiles per chunk
    assert QC == SC and NQC == NSC

    nc = bacc.Bacc("TRN2", target_bir_lowering=False, debug=False,
                   num_devices=n_cores)

    xT = nc.dram_tensor("xT", [H, S], BF16, kind="ExternalInput").ap()
    wqT = nc.dram_tensor("wqT", [H, M], BF16, kind="ExternalInput").ap()
    wkT = nc.dram_tensor("wkT", [H, M], BF16, kind="ExternalInput").ap()
    wvT = nc.dram_tensor("wvT", [H, M], BF16, kind="ExternalInput").ap()
    woT = nc.dram_tensor("woT", [M, H], BF16, kind="ExternalInput").ap()
    trig = nc.dram_tensor("trig", [P, 2, S], F32, kind="ExternalInput").ap()
    if causal:
        diag_d = nc.dram_tensor("diag", [P, DJ, SC], BF16,
                                kind="ExternalInput").ap()
    else:
        maskT_d = nc.dram_tensor("maskT", [S, S], F32, kind="ExternalInput").ap()
    outp = nc.dram_tensor("outp", [S, H], F32, kind="ExternalOutput").ap()

    xT_r = xT.rearrange("(ko p) s -> p ko s", p=P)
    wq_r = wqT.rearrange("(ko p) m -> p ko m", p=P)
    wk_r = wkT.rearrange("(ko p) m -> p ko m", p=P)
    wv_r = wvT.rearrange("(ko p) m -> p ko m", p=P)
    wo_r = woT.rearrange("(mt p) o -> p mt o", p=P)

    with tile.TileContext(nc) as tc, ExitStack() as top, \
         nc.allow_low_precision(reason="bf16 operands; fp32 PSUM accumulate"):
        persist = top.enter_context(tc.tile_pool(name="persist", bufs=1))
        ps_mm = top.enter_context(tc.tile_pool(name="ps_mm", bufs=5, space="PSUM"))
        ps_pv = top.enter_context(tc.tile_pool(name="ps_pv", bufs=2, space="PSUM"))
        ps_sm = top.enter_context(tc.tile_pool(name="ps_sm", bufs=1, space="PSUM"))

        qT_sb = persist.tile([P, NMT, S], BF16)   # [d, head, s]
        kT_sb = persist.tile([P, NMT, S], BF16)
        v_sb = persist.tile([P, NST, M], BF16)    # [sk%128, sk//128, m]
        ones_sb = persist.tile([P, P], BF16)      # lhsT for column sums
        if causal:
            diag_sb = persist.tile([P, DJ, SC], BF16)

        for _rep in range(repeat):
            nc.vector.memset(ones_sb[:], 1.0)

            with tc.tile_pool(name="wq", bufs=1) as wqp, \
                 tc.tile_pool(name="wk", bufs=1) as wkp, \
                 tc.tile_pool(name="wv", bufs=1) as wvp, \
                 tc.tile_pool(name="xc", bufs=2) as xcp, \
                 tc.tile_pool(name="trig", bufs=1) as trigp, \
                 tc.tile_pool(name="ropetmp", bufs=1) as rtp, \
                 tc.tile_pool(name="wo", bufs=1) as wop, \
                 tc.tile_pool(name="expq", bufs=2) as epq, \
                 tc.tile_pool(name="exps", bufs=4) as eps, \
                 tc.tile_pool(name="smx", bufs=2) as smp, \
                 tc.tile_pool(name="attc", bufs=2) as attp, \
                 tc.tile_pool(name="oout", bufs=2) as oop, \
                 (tc.tile_pool(name="mskt", bufs=3) if not causal
                  else ExitStack()) as mtp:

                wq_sb = wqp.tile([P, NKO, M], BF16)
                wk_sb = wkp.tile([P, NKO, M], BF16)
                wv_sb = wvp.tile([P, NKO, M], BF16)
                wo_sb = wop.tile([P, NMT, H], BF16)
                xcs = {}
                trigs = {}

                def load_trig(c):
                    trigs[c] = trigp.tile([P, 2, QC], F32, tag="trig", name="tg")
                    nc.sync.dma_start(trigs[c][:], trig[:, :, c * QC:(c + 1) * QC])

                def load_xc(c, engine=None):
                    xcs[c] = xcp.tile([P, NKO, QC], BF16, tag="xc", name="xc")
                    (engine or nc.sync).dma_start(
                        xcs[c][:], xT_r[:, :, c * QC:(c + 1) * QC])
                    if c not in trigs:
                        load_trig(c)

                def qk_proj(c):
                    tg = trigs.pop(c)
                    cs = slice(c * QC, (c + 1) * QC)
                    for w_sb, dst in ((wq_sb, qT_sb), (wk_sb, kT_sb)):
                        for mt in range(NMT):
                            ps = ps_mm.tile([P, QC], F32, tag="mm", name="ps_qk")
                            for ko in range(NKO):
                                nc.tensor.matmul(
                                    ps, w_sb[:, ko, mt * P:(mt + 1) * P],
                                    xcs[c][:, ko, :],
                                    start=(ko == 0), stop=(ko == NKO - 1))
                            # RoPE: out = ps*cos + swap64(ps)*sin_signed
                            t1 = rtp.tile([P, QC], F32, tag="t1")
                            nc.vector.tensor_tensor(t1[:], ps, tg[:, 0, :], MULT)
                            t2 = rtp.tile([P, QC], F32, tag="t2")
                            nc.vector.tensor_tensor(t2[0:64, :], ps[64:128, :],
                                                    tg[0:64, 1, :], MULT)
                            nc.vector.tensor_tensor(t2[64:128, :], ps[0:64, :],
                                                    tg[64:128, 1, :], MULT)
                            nc.vector.tensor_add(dst[:, mt, cs], t1[:], t2[:])

                def v_proj(c):
                    for st in range(QC // P):
                        ps = ps_mm.tile([P, M], F32, tag="mm", name="ps_v")
                        for ko in range(NKO):
                            nc.tensor.matmul(
                                ps, xcs[c][:, ko, st * P:(st + 1) * P],
                                wv_sb[:, ko, :],
                                start=(ko == 0), stop=(ko == NKO - 1))
                        nc.scalar.activation(
                            v_sb[:, c * (QC // P) + st, :], ps, COPY)

                def attn_chunk(c):
                    cq = slice(c * SC, (c + 1) * SC)
                    nt = min(NST, (c + 1) * SC // P) if causal else NST
                    attn_c = attp.tile([P, NMT, SC], BF16, tag="attc",
                                       name="attn_c")
                    for h in range(NMT):
                        hs = slice(h * P, (h + 1) * P)
                        pv = ps_pv.tile([P, SC], F32, tag="pv")
                        sm = ps_sm.tile([P, SC], F32, tag="sums")
                        ndiag = c * DJ if (causal and combine) else 0
                        first_sum = [True]

                        def issue_scores(t, e_dst):
                            j = t - c * DJ
                            off = j * P if (causal and j >= 0) else 0
                            ss_full = ps_mm.tile([P, SC], F32, tag="mm")
                            ss = ss_full[:, off:]
                            nc.tensor.matmul(
                                ss, kT_sb[:, h, t * P:(t + 1) * P],
                                qT_sb[:, h, c * SC + off:(c + 1) * SC],
                                start=True, stop=True)
                            if causal:
                                if j >= 0:
                                    we = (j + 1) * P
                                    nc.vector.tensor_add(
                                        ss_full[:, off:we], ss_full[:, off:we],
                                        diag_sb[:, j, off:we])
                            else:
                                mt_t = mtp.tile([P, SC], F32, tag="mask")
                                nc.sync.dma_start(
                                    mt_t[:], maskT_d[t * P:(t + 1) * P, cq])
                                nc.vector.tensor_add(ss, ss, mt_t[:, off:])
                                nc.vector.tensor_scalar_max(ss, ss, -3.0e38)
                            nc.scalar.activation(e_dst[:, off:], ss, EXP)
                            return (t, off, e_dst)

                        def flush_pv(t, off, e):
                            nc.tensor.matmul(
                                pv[:, off:], v_sb[:, t, hs], e[:, off:],
                                start=(t == 0), stop=(t == nt - 1),
                                skip_group_check=True)

                        def issue_sum(off, rhs, is_last):
                            nc.tensor.matmul(
                                sm[:, off:], ones_sb[:], rhs[:, off:],
                                start=first_sum[0], stop=is_last,
                                skip_group_check=True)
                            first_sum[0] = False

                        pend = []
                        sums_ready = []
                        equad = None
                        for t in range(nt):
                            if t < ndiag:
                                gi = t % 4
                                if gi == 0:
                                    equad = epq.tile([P, 4, SC], BF16,
                                                     tag="e4")
                                pend.append(issue_scores(t, equad[:, gi, :]))
                                if gi == 3:
                                    e2 = epq.tile([P, 2, SC], BF16, tag="e2")
                                    nc.vector.tensor_add(
                                        e2[:], equad[:, 0:2, :],
                                        equad[:, 2:4, :])
                                    e1 = epq.tile([P, SC], BF16, tag="e1")
                                    nc.vector.tensor_add(
                                        e1[:], e2[:, 0, :], e2[:, 1, :])
                                    sums_ready.append((0, e1, False))
                            else:
                                e = eps.tile([P, SC], BF16, tag="e")
                                res = issue_scores(t, e)
                                pend.append(res)
                                sums_ready.append((res[1], e, t == nt - 1))
                            if len(pend) > 3:
                                flush_pv(*pend.pop(0))
                            if len(sums_ready) > 1:
                                issue_sum(*sums_ready.pop(0))
                        for args in pend:
                            flush_pv(*args)
                        for s in sums_ready:
                            issue_sum(*s)

                        rec = smp.tile([P, SC], F32, tag="rec")
                        nc.vector.reciprocal(rec[:], sm)
                        nc.vector.tensor_tensor(attn_c[:, h, :], pv, rec[:],
                                                MULT)

                    # o-proj for this sq chunk (all heads of attn_c ready)
                    for st in range(SC // P):
                        rs = slice(c * SC + st * P, c * SC + (st + 1) * P)
                        o_st = oop.tile([P, H], F32, tag="o_st", name="o_st")
                        for oc in range(H // SC):
                            ps = ps_mm.tile([P, SC], F32, tag="mm", name="ps_o")
                            for mt in range(NMT):
                                nc.tensor.matmul(
                                    ps, attn_c[:, mt, st * P:(st + 1) * P],
                                    wo_sb[:, mt, oc * SC:(oc + 1) * SC],
                                    start=(mt == 0), stop=(mt == NMT - 1))
                            nc.scalar.activation(
                                o_st[:, oc * SC:(oc + 1) * SC], ps, COPY)
                            if oc % 2 == 1:  # flush staged half early
                                hs_o = slice((oc - 1) * SC, (oc + 1) * SC)
                                nc.sync.dma_start(outp[rs, hs_o],
                                                  o_st[:, hs_o])

                # DMA order: x chunk 0, wq per head-slice (first q chain can
                # start early), trig0, wk, then wv/wo + x1+ stream under
                # compute.
                load_xc(0)
                for mt in range(NMT):
                    ms = slice(mt * P, (mt + 1) * P)
                    nc.sync.dma_start(wq_sb[:, :, ms], wq_r[:, :, ms])
                load_trig(0)
                for mt in range(NMT):
                    ms = slice(mt * P, (mt + 1) * P)
                    nc.sync.dma_start(wk_sb[:, :, ms], wk_r[:, :, ms])
                if causal:
                    nc.scalar.dma_start(diag_sb[:], diag_d[:])

                # c=0/1 front: q,k first (wv still in flight), then v(0), v(1)
                qk_proj(0)
                load_xc(1)
                nc.sync.dma_start(wv_sb[:], wv_r)
                qk_proj(1)
                v_proj(0)
                v_proj(1)
                del xcs[0]
                for mt in range(NMT):
                    nc.scalar.dma_start(wo_sb[:, mt], wo_r[:, mt])

                if fuse:
                    # attn chunk c right after its projections; projections of
                    # chunk c+1 interleave with attention/o-proj of chunk c-1.
                    attn_chunk(0)
                    for c in range(2, NQC):
                        load_xc(c)
                        qk_proj(c)
                        v_proj(c)
                        del xcs[c - 1]
                        attn_chunk(c - 1)
                    attn_chunk(NQC - 1)
                else:
                    for c in range(2, NQC):
                        load_xc(c)
                        qk_proj(c)
                        v_proj(c)
                        del xcs[c - 1]
                    for c in range(NSC):
                        attn_chunk(c)

    nc.compile()
    return nc


_CACHE = {}


def _get_program(S, H, M, causal, repeat=1, combine=True, fuse=True):
    key = (S, H, M, causal, repeat, combine, fuse)
    if key not in _CACHE:
        _CACHE[key] = _build(S, H, M, causal, repeat=repeat, combine=combine,
                             fuse=fuse)
    return _CACHE[key]


def _rope_tables(S, dim, base=10000.0):
    inv_freq = 1.0 / (base ** (np.arange(0, dim, 2, dtype=np.float64) / dim))
    t = np.arange(S, dtype=np.float64)
    freqs = np.outer(t, inv_freq)                     # [S, dim/2]
    emb = np.concatenate([freqs, freqs], axis=-1)     # [S, dim]
    return (np.cos(emb).astype(np.float32), np.sin(emb).astype(np.float32))


def _prep_in_maps(hidden_states, attention_mask, position_ids,
                  wq, wk, wv, wo):
    """Host-side shard + cast. Returns (in_maps, causal, M)."""
    Bq, S, H = hidden_states.shape
    assert Bq == B and H % HD == 0
    nh = H // HD
    groups = NCORES // B                     # head-groups per batch
    hpg = nh // groups                       # heads per core
    M = hpg * HD

    bf = ml_dtypes.bfloat16

    # causal-mask detection (exact match against the standard Llama pattern)
    neg = np.finfo(np.float32).min
    causal_ref = np.where(np.tril(np.ones((S, S), dtype=bool)),
                          np.float32(0.0), np.float32(neg))
    causal = all(np.array_equal(attention_mask[b, 0], causal_ref)
                 for b in range(B))

    cos_tab, sin_tab = _rope_tables(S, HD)
    scale = 1.0 / np.sqrt(HD)

    SC = 512
    P = 128
    in_maps = []
    for core in range(NCORES):
        b, g = divmod(core, groups)
        rows = slice(g * M, (g + 1) * M)
        x = hidden_states[b]                                   # [S, H]
        pos = position_ids[b].astype(np.int64)
        cosT = cos_tab[pos].T                                  # [HD, S]
        sinT = sin_tab[pos].T
        sinS = np.concatenate([-sinT[:HD // 2], sinT[HD // 2:]], axis=0)
        m = {
            "xT": np.ascontiguousarray(x.T).astype(bf),
            "wqT": np.ascontiguousarray((wq[rows].astype(np.float64) * scale)
                                        .astype(np.float32).T).astype(bf),
            "wkT": np.ascontiguousarray(wk[rows].T).astype(bf),
            "wvT": np.ascontiguousarray(wv[rows].T).astype(bf),
            "woT": np.ascontiguousarray(wo[:, rows].T).astype(bf),
            "trig": np.ascontiguousarray(
                np.stack([cosT, sinS], axis=1).astype(np.float32)),
        }
        if causal:
            p_idx = np.arange(P)[:, None, None]
            j_idx = np.arange(SC // P)[None, :, None]
            f_idx = np.arange(SC)[None, None, :]
            m["diag"] = np.where(p_idx + P * j_idx <= f_idx,
                                 np.float32(0.0),
                                 np.float32(NEG)).astype(bf)
        else:
            m["maskT"] = np.ascontiguousarray(attention_mask[b, 0].T)
        in_maps.append(m)
    return in_maps, causal, M


def kernel(hidden_states, attention_mask, position_ids, wq, wk, wv, wo):
    global LAST_RESULTS
    hidden_states = np.asarray(hidden_states, dtype=np.float32)
    attention_mask = np.asarray(attention_mask, dtype=np.float32)
    position_ids = np.asarray(position_ids)
    wq = np.asarray(wq, dtype=np.float32)
    wk = np.asarray(wk, dtype=np.float32)
    wv = np.asarray(wv, dtype=np.float32)
    wo = np.asarray(wo, dtype=np.float32)

    Bq, S, H = hidden_states.shape
    in_maps, causal, M = _prep_in_maps(
        hidden_states, attention_mask, position_ids, wq, wk, wv, wo)

    nc = _get_program(S, H, M, causal)
    globals()["LAST_IN_MAPS"] = in_maps
    res = run_bass_kernel_spmd(nc, in_maps, core_ids=list(range(NCORES)),
                               trace=False)
    LAST_RESULTS = res

    groups = NCORES // B
    out = np.zeros((B, S, H), dtype=np.float64)
    for core in range(NCORES):
        b = core // groups
        out[b] += res.results[core]["outp"].astype(np.float64)
    return out.astype(np.float32)
